# revision 4
# baseline (speedup 1.0000x reference)
"""Trainium2 Bass kernel for nn_EnhancedGNNTransformerEncoder (4-layer
TransformerConv GNN + mean-pool + linear head).

Sharding: destination nodes (and their incident edges) are split across the
8 NeuronCores; the small weight matrices are replicated.  Per layer each core
computes K/V rows for its own shard, AllGathers them (overlapped with the
Q/S projection pass), then per 128-dst-node window gathers per-edge K/V/Q
rows by DMA, computes the segment softmax with a pairwise-add tree, and
aggregates with host-precomputed one-hot selector matmuls on the PE.

Host-side tricks: 1/sqrt(C) folded into Wq; V/skip stored head-transposed
(c-major) so the per-head softmax weight broadcasts without materialization;
padded edge slots select nothing (all-zero selector column) so no mask is
needed; all gather indices preloaded into SBUF once.
"""

import os
import sys
import types

import numpy as np
import ml_dtypes

# ---------------------------------------------------------------------------
# NTFF profile hook (absent antenv.axon_hooks on this image) so trace=True
# works under axon.
if "antenv.axon_hooks" not in sys.modules:
    _m = types.ModuleType("antenv.axon_hooks")
    _m._hook = None

    def _set(h):
        _m._hook = h

    def _get():
        return _m._hook

    _m.set_axon_ntff_profile_hook = _set
    _m.get_axon_ntff_profile_hook = _get
    sys.modules["antenv.axon_hooks"] = _m
    try:
        import antenv

        antenv.axon_hooks = _m
    except Exception:
        pass
    try:
        from trn_agent_boot.trn_boot import _ntff_profile_via_ctypes

        _m._hook = _ntff_profile_via_ctypes("/opt/axon/libaxon_pjrt.so")
    except Exception:
        pass

import concourse.bass as bass
import concourse.mybir as mybir
import concourse.tile as tile
from concourse import bacc
from concourse import bass_utils
from concourse.masks import make_identity

F16 = mybir.dt.float16
BF16 = mybir.dt.bfloat16
F32 = mybir.dt.float32
I16 = mybir.dt.int16
AX = mybir.AxisListType
OP = mybir.AluOpType
ACTF = mybir.ActivationFunctionType

# problem constants (hardcoded per the harness contract)
N, E, IN, H, C, G, OUT = 50000, 800000, 128, 8, 32, 64, 64
HC = H * C  # 256
NLAYERS = 4
NC = 8
NLOC = N // NC          # 6250
W = 50                  # windows of 128 dst rows per core
NPAD = W * 128          # 6400 local rows (incl dummies)
NPAD_ALL = NC * NPAD    # 51200 kv rows
HALF = NPAD_ALL // 2    # 25600 (int16-addressable half)
SCALE = float(1.0 / np.sqrt(C))

_BUILD_CACHE = {}
LAST_RESULT = None


def _build(ewl, ewh, run_layers, use_bias=True):
    """Build + compile the SPMD program.  ewl/ewh: padded lo/hi edge slots
    per window (multiples of 128)."""
    skip_b = bool(int(os.environ.get("KSKIP_B", "0")))
    skip_c = bool(int(os.environ.get("KSKIP_C", "0")))
    nwin = int(os.environ.get("KWIN", str(W)))
    key = (ewl, ewh, run_layers, skip_b, skip_c, nwin, use_bias)
    if key in _BUILD_CACHE:
        return _BUILD_CACHE[key]

    S = (ewl + ewh) // 128          # kv slots per window
    SL, SH = ewl // 128, ewh // 128
    EW = ewl + ewh
    CW = (EW + EW) // 16            # idx cols per window (lo+hi+q)

    nc = bacc.Bacc("TRN2", target_bir_lowering=False, debug=False,
                   enable_asserts=False, num_devices=NC,
                   num_swdge_queues=4)

    # ---- external inputs (per-core content, same shapes) ----
    xT_loc = nc.dram_tensor("xT_loc", [128, NPAD], F16, kind="ExternalInput")
    w_all = nc.dram_tensor("w_all", [128, NLAYERS, 2, 4 * HC], F16, kind="ExternalInput")
    b_all = nc.dram_tensor("b_all", [1, NLAYERS, 4 * HC], F16, kind="ExternalInput")
    ones1 = nc.dram_tensor("ones1", [1, 128], F16, kind="ExternalInput")
    w_fc = nc.dram_tensor("w_fc", [128, 2, OUT], F16, kind="ExternalInput")
    b_fc = nc.dram_tensor("b_fc", [G, OUT], F32, kind="ExternalInput")
    idx_all = nc.dram_tensor("idx_all", [128, W * CW], I16, kind="ExternalInput")
    sel_hot = nc.dram_tensor("sel_hot", [128, W * S * 128], BF16,
                             kind="ExternalInput")
    gsel = nc.dram_tensor("gsel", [128, W * G], F16, kind="ExternalInput")

    out_d = nc.dram_tensor("out", [G, OUT], F32, kind="ExternalOutput")

    # ---- internal DRAM ----
    kv_loc = nc.dram_tensor("kv_loc", [NPAD, 2 * HC], F16, kind="Internal")
    kv_full = nc.dram_tensor("kv_full", [NPAD_ALL, 2 * HC], F16, kind="Internal",
                             addr_space="Shared")
    qskip_full = nc.dram_tensor("qskip_full", [NPAD, 2 * HC], F16, kind="Internal")
    hT_shard = nc.dram_tensor("hT_shard", [2, 128, NPAD], F16, kind="Internal")
    pool_part = nc.dram_tensor("pool_part", [2, 128, OUT], F32, kind="Internal")
    pool_sum = nc.dram_tensor("pool_sum", [2, 128, OUT], F32, kind="Internal",
                              addr_space="Shared")

    SLAB = 1280         # 10 node-tiles per slab
    NSLAB = NPAD // SLAB  # 5

    with tile.TileContext(nc) as tc:
        with tc.tile_pool(name="const", bufs=1) as cp, \
             tc.tile_pool(name="slab", bufs=2) as slabp, \
             tc.tile_pool(name="kvb", bufs=2) as kvbp, \
             tc.tile_pool(name="win", bufs=2) as winp, \
             tc.tile_pool(name="psA", bufs=2, space="PSUM") as psA, \
             tc.tile_pool(name="psB", bufs=2, space="PSUM") as psB, \
             tc.tile_pool(name="psT", bufs=2, space="PSUM") as psT:

            # ---- load constants ----
            wall_sb = cp.tile([128, NLAYERS, 2, 4 * HC], F16)
            ball_sb = cp.tile([1, NLAYERS, 4 * HC], F16)
            ones_sb = cp.tile([1, 128], F16)
            wfc_sb = cp.tile([128, 2, OUT], F16)
            bfc_sb = cp.tile([G, OUT], F32)
            idx_sb = cp.tile([128, W * CW], I16)
            gsel_sb = cp.tile([128, W * G], F16)
            ident_sb = cp.tile([128, 128], F16)
            pool_acc = cp.tile([128, 2, OUT], F32)

            for t, d in [(wall_sb, w_all), (ball_sb, b_all),
                         (ones_sb, ones1), (wfc_sb, w_fc),
                         (bfc_sb, b_fc), (idx_sb, idx_all),
                         (gsel_sb, gsel)]:
                nc.sync.dma_start(out=t[:], in_=d.ap())
            make_identity(nc, ident_sb[:])

            qctr = [0]

            def gather_chunked(out_tile, slot0, in_ap, col0, n, elem,
                               estep=None):
                done = 0
                while done < n:
                    cur = min(1024, n - done)
                    nc.gpsimd.dma_gather(
                        out_ap=out_tile[:, slot0 + done // 128:
                                        slot0 + (done + cur) // 128, :],
                        in_ap=in_ap,
                        idxs_ap=idx_sb[:, col0 + done // 16:
                                       col0 + (done + cur) // 16],
                        num_idxs=cur, num_idxs_reg=cur, elem_size=elem,
                        elem_step=estep, single_packet=True,
                        queue_num=qctr[0] % 4)
                    qctr[0] += 1
                    done += cur

            for layer in range(run_layers):
                KH = 1 if layer == 0 else 2
                last = layer == NLAYERS - 1

                # ===== Phase A: two passes over the shard: KV, then QS.
                # The kv AllGather is issued between them so it overlaps the
                # QS projection (and the first windows' q gathers / loads).
                for part in range(2):           # 0: K|V -> kv_loc, 1: Q|S
                    dst_t = kv_loc if part == 0 else qskip_full
                    pcs = slice(part * 2 * HC, (part + 1) * 2 * HC)
                    for sl in range(NSLAB):
                        slabs = []
                        for kh in range(KH):
                            st = slabp.tile([128, SLAB], F16, tag="slab",
                                            bufs=4)
                            if layer == 0:
                                src_ap = xT_loc.ap()[:, sl * SLAB:(sl + 1) * SLAB]
                            else:
                                src_ap = hT_shard.ap()[kh, :,
                                                       sl * SLAB:(sl + 1) * SLAB]
                            nc.sync.dma_start(out=st[:], in_=src_ap)
                            slabs.append(st)
                        for half5 in range(2):
                            buf = kvbp.tile([128, 5, 2 * HC], F16, tag="kvb",
                                            bufs=3)
                            for j5 in range(5):
                                j = half5 * 5 + j5
                                ps = psA.tile([128, 2 * HC], F32, tag="psA")
                                for kh in range(KH):
                                    nc.tensor.matmul(
                                        ps[:],
                                        lhsT=slabs[kh][:, j * 128:(j + 1) * 128],
                                        rhs=wall_sb[:, layer, kh, pcs],
                                        start=(kh == 0),
                                        stop=(not use_bias and kh == KH - 1),
                                        skip_group_check=True)
                                if use_bias:
                                    nc.tensor.matmul(
                                        ps[:], lhsT=ones_sb[:],
                                        rhs=ball_sb[:, layer, pcs],
                                        start=False, stop=True,
                                        skip_group_check=True)
                                if j % 2 == 0:
                                    nc.vector.tensor_copy(buf[:, j5, :], ps[:])
                                else:
                                    nc.scalar.activation(buf[:, j5, :], ps[:],
                                                         ACTF.Copy)
                            base = sl * SLAB + half5 * 640
                            dst_ap = dst_t.ap()[base:base + 640, :]
                            nc.sync.dma_start(
                                out=dst_ap.rearrange("(t p) e -> p t e", p=128),
                                in_=buf[:])
                    if part == 0:
                        if not skip_c:
                            nc.gpsimd.collective_compute(
                                "AllGather", OP.bypass,
                                replica_groups=[list(range(NC))],
                                ins=[kv_loc.ap()], outs=[kv_full.ap()])
                        else:
                            nc.sync.dma_start(
                                out=kv_full.ap()[0:NPAD, :], in_=kv_loc.ap())

                # ===== Phase B: windows =====
                if skip_b:
                    continue
                if last:
                    nc.vector.memset(pool_acc[:], 0.0)

                for w in range(nwin):
                    cb = w * CW
                    # per-window loads (independent of the AllGather)
                    selw = winp.tile([128, S, 128], BF16, tag="selw", bufs=2)
                    nc.sync.dma_start(
                        out=selw[:],
                        in_=sel_hot.ap()[:, w * S * 128:(w + 1) * S * 128])
                    qs_w = winp.tile([128, 2 * HC], F16, tag="qsw", bufs=2)
                    nc.sync.dma_start(
                        out=qs_w[:],
                        in_=qskip_full.ap()[w * 128:(w + 1) * 128, :])
                    q_t = winp.tile([128, S, HC], F16, tag="qt", bufs=2)
                    gather_chunked(q_t, 0, qskip_full.ap()[:, 0:HC],
                                   cb + EW // 16, EW, HC, estep=2 * HC)
                    # kv gathers (wait on the AllGather)
                    kv_t = winp.tile([128, S, 2 * HC], F16, tag="kvt", bufs=3)
                    gather_chunked(kv_t, 0, kv_full.ap()[0:HALF, :],
                                   cb, ewl, 2 * HC)
                    gather_chunked(kv_t, SL, kv_full.ap()[HALF:NPAD_ALL, :],
                                   cb + ewl // 16, ewh, 2 * HC)

                    # logits: per-edge q.k, pairwise-add tree over C
                    qk = winp.tile([128, S, H, C], F16, tag="qk", bufs=1)
                    nc.vector.tensor_tensor(
                        qk[:],
                        q_t[:].rearrange("p s (h c) -> p s h c", c=C),
                        kv_t[:, :, 0:HC].rearrange("p s (h c) -> p s h c", c=C),
                        OP.mult)
                    t16 = winp.tile([128, S, H, 16], F16, tag="t16", bufs=1)
                    nc.vector.tensor_tensor(t16[:], qk[:, :, :, 0:16],
                                            qk[:, :, :, 16:32], OP.add)
                    t8 = winp.tile([128, S, H, 8], F16, tag="t8", bufs=1)
                    nc.vector.tensor_tensor(t8[:], t16[:, :, :, 0:8],
                                            t16[:, :, :, 8:16], OP.add)
                    t4 = winp.tile([128, S, H, 4], F16, tag="t4", bufs=1)
                    nc.vector.tensor_tensor(t4[:], t8[:, :, :, 0:4],
                                            t8[:, :, :, 4:8], OP.add)
                    t2 = winp.tile([128, S, H, 2], F16, tag="t2", bufs=1)
                    nc.vector.tensor_tensor(t2[:], t4[:, :, :, 0:2],
                                            t4[:, :, :, 2:4], OP.add)
                    logits = winp.tile([128, S, H], F32, tag="lg", bufs=2)
                    nc.vector.tensor_tensor(logits[:], t2[:, :, :, 0],
                                            t2[:, :, :, 1], OP.add)

                    # p = exp(logits) (1/sqrt(C) folded into Wq on host;
                    # padded slots are dropped by the all-zero sel column)
                    p_bf = winp.tile([128, S, H], BF16, tag="pbf", bufs=2)
                    nc.scalar.activation(p_bf[:], logits[:], ACTF.Exp)

                    # wv: V is stored c-major so p broadcasts per head
                    wv = winp.tile([128, S, HC + H], BF16, tag="wv", bufs=2)
                    nc.vector.tensor_tensor(
                        wv[:, :, 0:HC].rearrange("p s (c h) -> p s c h", h=H),
                        kv_t[:, :, HC:2 * HC].rearrange("p s (c h) -> p s c h",
                                                        h=H),
                        p_bf[:, :, None, :].to_broadcast((128, S, C, H)),
                        OP.mult)
                    nc.scalar.activation(wv[:, :, HC:HC + H], logits[:],
                                         ACTF.Exp)

                    # aggregate over edges via one-hot selector matmuls
                    agg = psB.tile([128, HC + H], F32, tag="agg")
                    for s in range(S):
                        nc.tensor.matmul(agg[:], lhsT=selw[:, s, :],
                                         rhs=wv[:, s, :],
                                         start=(s == 0), stop=(s == S - 1),
                                         skip_group_check=True)

                    # epilogue: normalize, skip, relu
                    rs0 = winp.tile([128, H], F32, tag="rs0", bufs=2)
                    nc.vector.tensor_scalar_add(rs0[:], agg[:, HC:HC + H],
                                                1e-16)
                    rs = winp.tile([128, H], F32, tag="rs", bufs=2)
                    nc.vector.reciprocal(rs[:], rs0[:])
                    tmp = winp.tile([128, HC], F32, tag="tmp", bufs=2)
                    nc.vector.tensor_tensor(
                        tmp[:].rearrange("p (c h) -> p c h", h=H),
                        agg[:, 0:HC].rearrange("p (c h) -> p c h", h=H),
                        rs[:, None, :].to_broadcast((128, C, H)),
                        OP.mult)
                    tmp2 = winp.tile([128, HC], F32, tag="tmp2", bufs=2)
                    nc.vector.tensor_tensor(tmp2[:], tmp[:],
                                            qs_w[:, HC:2 * HC], OP.add)
                    h_nm = winp.tile([128, HC], F16, tag="hnm", bufs=2)
                    nc.scalar.activation(h_nm[:], tmp2[:], ACTF.Relu)

                    if last:
                        for kh in range(2):
                            ptmp = psT.tile([128, OUT], F32, tag="trp",
                                            name="ptmp")
                            nc.tensor.matmul(
                                ptmp[:],
                                lhsT=h_nm[:, kh * 128:(kh + 1) * 128],
                                rhs=gsel_sb[:, w * G:(w + 1) * G],
                                start=True, stop=True,
                                skip_group_check=True)
                            nc.vector.tensor_tensor(
                                pool_acc[:, kh, :], pool_acc[:, kh, :],
                                ptmp[:], OP.add)
                    else:
                        hstage = winp.tile([128, 2, 128], F16, tag="hst",
                                           bufs=2)
                        for kh in range(2):
                            trp = psT.tile([128, 128], F16, tag="trp")
                            nc.tensor.transpose(
                                trp[:], h_nm[:, kh * 128:(kh + 1) * 128],
                                ident_sb[:])
                            if kh == 0:
                                nc.vector.tensor_copy(hstage[:, kh, :], trp[:])
                            else:
                                nc.scalar.activation(hstage[:, kh, :], trp[:],
                                                     ACTF.Copy)
                        nc.sync.dma_start(
                            out=hT_shard.ap().rearrange("k p n -> p k n")[
                                :, :, w * 128:(w + 1) * 128],
                            in_=hstage[:])

                # ===== Phase C =====
                if last:
                    nc.sync.dma_start(
                        out=pool_part.ap().rearrange("k p o -> p k o"),
                        in_=pool_acc[:])
                    if not skip_c:
                        nc.gpsimd.collective_compute(
                            "AllReduce", OP.add,
                            replica_groups=[list(range(NC))],
                            ins=[pool_part.ap()], outs=[pool_sum.ap()])
                    pooled = cp.tile([128, 2, OUT], F32)
                    nc.sync.dma_start(
                        out=pooled[:],
                        in_=pool_sum.ap().rearrange("k p o -> p k o"))
                    pooled16 = cp.tile([128, 2, OUT], F16)
                    nc.vector.tensor_copy(pooled16[:], pooled[:])
                    fin = psB.tile([G, OUT], F32, tag="agg")
                    for kh in range(2):
                        nc.tensor.matmul(fin[:], lhsT=pooled16[:, kh, :],
                                         rhs=wfc_sb[:, kh, :],
                                         start=(kh == 0), stop=(kh == 1))
                    out_sb = cp.tile([G, OUT], F32)
                    nc.vector.tensor_tensor(out_sb[:], fin[:], bfc_sb[:], OP.add)
                    nc.sync.dma_start(out=out_d.ap(), in_=out_sb[:])

            if run_layers < NLAYERS:
                # partial build (debug): emit output anyway so run works
                out_sb2 = cp.tile([G, OUT], F32)
                nc.vector.memset(out_sb2[:], 0.0)
                nc.sync.dma_start(out=out_d.ap(), in_=out_sb2[:])

    nc.compile()
    _BUILD_CACHE[key] = nc
    return nc


def _wrap16_one(a):
    """[n] int array -> [16, n//16] gather-index layout (one window piece)."""
    return a.reshape(-1, 16).T


def _host_prep(inputs):
    x = np.asarray(inputs["x"], np.float32)
    ei = np.asarray(inputs["edge_index"]).astype(np.int64)
    batch = np.asarray(inputs["batch"]).astype(np.int64)
    src, dst = ei[0], ei[1]

    f16 = np.float16
    bf16 = ml_dtypes.bfloat16

    # feature permutation h*C+c -> c*H+h (c-major) for the V/skip path
    perm_cm = np.arange(HC).reshape(H, C).T.reshape(-1)  # [c*H+h] = h*C+c

    def getf(name):
        return np.asarray(inputs[name], np.float32)

    # per-layer transformed weights (row-permute inputs for layers > 0
    # because the hidden state is c-major; fold 1/sqrt(C) into Wq;
    # column-permute V and S outputs to c-major)
    Wk_l, Wv_l, Wq_l, Ws_l = [], [], [], []
    bk_l, bv_l, bq_l, bs_l = [], [], [], []
    for l in range(NLAYERS):
        if l == 0:
            Wk, Wv, Wq, Ws = getf("Wk0"), getf("Wv0"), getf("Wq0"), getf("Ws0")
            bk, bv, bq, bs = getf("bk0"), getf("bv0"), getf("bq0"), getf("bs0")
        else:
            Wk, Wv = getf("Wk")[l - 1], getf("Wv")[l - 1]
            Wq, Ws = getf("Wq")[l - 1], getf("Ws")[l - 1]
            bk, bv = getf("bk")[l - 1], getf("bv")[l - 1]
            bq, bs = getf("bq")[l - 1], getf("bs")[l - 1]
            Wk, Wv, Wq, Ws = (Wk[perm_cm], Wv[perm_cm], Wq[perm_cm],
                              Ws[perm_cm])
        Wk_l.append(Wk); bk_l.append(bk)
        Wv_l.append(Wv[:, perm_cm]); bv_l.append(bv[perm_cm])
        Wq_l.append(Wq * SCALE); bq_l.append(bq * SCALE)
        Ws_l.append(Ws[:, perm_cm]); bs_l.append(bs[perm_cm])

    def pack_w(Wa_l, Wb_l):
        w = np.zeros((128, NLAYERS, 2, 2 * HC), f16)
        for l in range(NLAYERS):
            a = Wa_l[l].astype(f16)
            b = Wb_l[l].astype(f16)
            kmax = 1 if l == 0 else 2
            for kh in range(kmax):
                w[:, l, kh, 0:HC] = a[kh * 128:(kh + 1) * 128]
                w[:, l, kh, HC:] = b[kh * 128:(kh + 1) * 128]
        return w

    wall = np.concatenate([pack_w(Wk_l, Wv_l), pack_w(Wq_l, Ws_l)], axis=3)

    def pack_b(ba_l, bb_l):
        b = np.zeros((1, NLAYERS, 2 * HC), f16)
        for l in range(NLAYERS):
            b[0, l, 0:HC] = ba_l[l].astype(f16)
            b[0, l, HC:] = bb_l[l].astype(f16)
        return b

    ball = np.concatenate([pack_b(bk_l, bv_l), pack_b(bq_l, bs_l)], axis=2)
    use_bias = bool(np.abs(ball).max() > 0)

    wfc = getf("Wfc")[perm_cm].astype(f16)
    wfc_p = np.ascontiguousarray(wfc.reshape(2, 128, OUT).transpose(1, 0, 2))
    bfc_rep = np.tile(np.asarray(inputs["bfc"], np.float32)[None, :], (G, 1))

    ones1 = np.ones((1, 128), f16)

    counts = np.bincount(batch, minlength=G).astype(np.float32)
    inv_counts = (1.0 / np.maximum(counts, 1.0)).astype(np.float32)

    # ---- balanced node->window assignment (per core) ----
    perms = []        # per core: original local idx -> padded local row
    core_edges = []
    for c in range(NC):
        m = (dst >= c * NLOC) & (dst < (c + 1) * NLOC)
        s_c = src[m]
        dloc = dst[m] - c * NLOC
        core_edges.append((s_c, dloc))
        lo_e = ((s_c // NLOC) * NPAD + (s_c % NLOC)) < HALF
        lod = np.bincount(dloc[lo_e], minlength=NLOC).astype(np.int64)
        hid = np.bincount(dloc[~lo_e], minlength=NLOC).astype(np.int64)
        order = np.argsort(-(lod + hid), kind="stable")
        wlo = np.zeros(W); whi = np.zeros(W); wcnt = np.zeros(W, np.int64)
        wof = np.empty(NLOC, np.int64)
        for n in order:
            cand = np.where(wcnt < 128)[0]
            score = np.maximum(wlo[cand] + lod[n], whi[cand] + hid[n])
            j = cand[int(np.argmin(score))]
            wof[n] = j
            wlo[j] += lod[n]; whi[j] += hid[n]; wcnt[j] += 1
        perm = np.empty(NLOC, np.int64)
        fill = np.zeros(W, np.int64)
        for n in range(NLOC):
            wn = wof[n]
            perm[n] = wn * 128 + fill[wn]
            fill[wn] += 1
        perms.append(perm)

    def row_of(nodes):
        c_of = nodes // NLOC
        r = np.empty(len(nodes), np.int64)
        for c in range(NC):
            mm = c_of == c
            r[mm] = c * NPAD + perms[c][nodes[mm] % NLOC]
        return r

    # -- x transposed, padded + permuted layout --
    xT = np.zeros((128, NPAD_ALL), f16)
    xt = np.ascontiguousarray(x.T.astype(f16))
    for c in range(NC):
        cols = c * NPAD + perms[c]
        xT[:, cols] = xt[:, c * NLOC:(c + 1) * NLOC]

    ewl_max = ewh_max = 0
    prepped = []
    for c in range(NC):
        s_c, dloc = core_edges[c]
        s_row = row_of(s_c)
        drow = perms[c][dloc]
        w_of = drow // 128
        is_lo = s_row < HALF
        order = np.lexsort((drow, ~is_lo, w_of))
        s_row, drow, w_of, is_lo = (s_row[order], drow[order],
                                    w_of[order], is_lo[order])
        nlo = np.bincount(w_of[is_lo], minlength=W)
        nhi = np.bincount(w_of[~is_lo], minlength=W)
        ewl_max = max(ewl_max, int(nlo.max()))
        ewh_max = max(ewh_max, int(nhi.max()))
        prepped.append((s_row, drow, w_of, is_lo, nlo, nhi))

    ewl = -(-ewl_max // 128) * 128
    ewh = -(-ewh_max // 128) * 128
    EW = ewl + ewh
    S = EW // 128

    pp_all = np.arange(EW) % 128
    ss_all = np.arange(EW) // 128

    in_maps = []
    shared = dict(w_all=wall, b_all=ball, ones1=ones1,
                  w_fc=wfc_p, b_fc=bfc_rep.astype(np.float32))
    for c in range(NC):
        s_row, drow, w_of, is_lo, nlo, nhi = prepped[c]
        ilo = np.zeros((W, ewl), np.int64)
        ihi = np.zeros((W, ewh), np.int64)
        dpos = np.zeros((W, EW), np.int64)
        valid = np.zeros((W, EW), bool)
        iq = np.zeros((W, EW), np.int64)
        wstart = np.searchsorted(w_of, np.arange(W))
        wend = np.searchsorted(w_of, np.arange(W) + 1)
        for w in range(W):
            a, b = int(wstart[w]), int(wend[w])
            k = int(nlo[w])
            ilo[w, :k] = s_row[a:a + k]
            dpos[w, :k] = drow[a:a + k] % 128
            valid[w, :k] = True
            iq[w, :k] = drow[a:a + k]
            nh = b - (a + k)
            ihi[w, :nh] = s_row[a + k:b] - HALF
            dpos[w, ewl:ewl + nh] = drow[a + k:b] % 128
            valid[w, ewl:ewl + nh] = True
            iq[w, ewl:ewl + nh] = drow[a + k:b]

        # per-window idx layout: [lo | hi | q] columns, 16-row wrapped
        cols = []
        for w in range(W):
            cols += [_wrap16_one(ilo[w]), _wrap16_one(ihi[w]),
                     _wrap16_one(iq[w])]
        idx_np = np.tile(np.concatenate(cols, axis=1), (8, 1)).astype(np.int16)

        # one-hot selector [128, W, S, 128]: edge at (w, slot s, partition p)
        # contributes to dst column dpos; padded slots select nothing
        sel_np = np.zeros((128, W, S, 128), np.float32)
        for w in range(W):
            v = valid[w]
            sel_np[pp_all[v], w, ss_all[v], dpos[w, v]] = 1.0
        sel_np = sel_np.reshape(128, W * S * 128).astype(bf16)

        gs_flat = np.zeros((NPAD, G), np.float32)
        orig = np.arange(NLOC)
        gn = batch[c * NLOC + orig]
        gs_flat[perms[c][orig], gn] = inv_counts[gn]
        gsel_in = np.ascontiguousarray(
            gs_flat.reshape(W, 128, G).transpose(1, 0, 2)
            .reshape(128, W * G).astype(f16))

        xT_loc = np.ascontiguousarray(xT[:, c * NPAD:(c + 1) * NPAD])
        in_maps.append(dict(
            shared,
            xT_loc=xT_loc,
            idx_all=idx_np,
            sel_hot=sel_np,
            gsel=gsel_in))
    return in_maps, ewl, ewh, use_bias


def kernel(**inputs):
    global LAST_RESULT
    in_maps, ewl, ewh, use_bias = _host_prep(inputs)
    run_layers = int(os.environ.get("RUN_LAYERS", str(NLAYERS)))
    nc = _build(ewl, ewh, run_layers, use_bias)
    trace = bool(int(os.environ.get("KTRACE", "0")))
    res = bass_utils.run_bass_kernel_spmd(
        nc, in_maps, core_ids=list(range(NC)), trace=trace)
    LAST_RESULT = res
    return res.results[0]["out"].astype(np.float32)
